# revision 50
# baseline (speedup 1.0000x reference)
"""Bahdanau self-attention kernel for Trainium2 (8 NeuronCores, Bass/Tile).

Math (per batch b):
  Wi = B @ W.T                                  [N, D]
  S[i, j]  = sum_d v[d] * tanh(Wi[i,d] + Wi[j,d])   (symmetric)
  A = softmax(S, axis=-1)
  C = A @ B

Shapes: B [4, 512, 128], W [128, 128], v [128].

Sharding: 8 cores; core c handles batch b = c // 2, query rows
q0 = (c % 2) * 256 .. q0 + 255.  Each core receives its batch's rows
ROTATED so that its 256 query rows are rows 0..255 of its local key
matrix (softmax and the attention-weighted sum are invariant to key
order).  W / v are replicated, so one SPMD program serves all cores.

Per-core pipeline (all layouts put D=128 on partitions):
  - wik[d, n] = (W @ Bk^T)[d, n]    via PE transposes + one matmul
  - for each query i: tanh arg is wik + wik[:, i] broadcast along free
    axis.  DVE tensor_scalar_add builds stacked inputs (STACK queries
    per ACT instruction to amortize the ~352-cycle ACT overhead), ACT
    applies tanh at 1 elem/cycle/lane.
  - v-reduction over d (partitions) via PE: lhsT is a shifted view of a
    [128, 256] buffer holding v at column 128 and zeros elsewhere, so
    lhsT(i)[:, m] = v * (m == i); 128 matmuls accumulate S rows into one
    [128, 512] PSUM tile.
  - softmax: DVE reduce_max(negate) -> ACT exp(S - max) with fused
    free-axis accumulate (row sums) -> DVE reciprocal.
  - C: PE-transpose E, then 4 accumulating matmuls against Bk, scale by
    the reciprocal row sums, DMA out.
"""

import numpy as np
from contextlib import ExitStack

import concourse.bacc as bacc
import concourse.mybir as mybir
import concourse.tile as tile
from concourse.bass_utils import run_bass_kernel_spmd
from concourse.masks import make_identity

F32 = mybir.dt.float32
F32R = mybir.dt.float32r
P = 128  # partitions == feature dim D
N = 512  # sequence length per batch
NB = 4  # batches
NCORES = 8
NQ = 256  # queries per core
NBLK = NQ // P  # query blocks of 128 per core
STACK = 16  # queries per ACT instruction

TRACE = False
LAST_RESULT = None  # BassKernelResults of the most recent run (for profiling)

_program = None


def _groups(ib):
    """Group schedule for block ib: list of (local_start, size).

    Sizes fill the STACK*N stack tile (more queries per ACT call as the
    ragged width shrinks), with a small ramp-in on block 0 (so the first
    ACT fires early) and a taper at the end of the last block (so the
    final PE burst before the last exp is short).  All starts/sizes even
    (fp32r matmul offsets must be even).
    """
    res = []
    q = 0
    if ib == 0:
        for s in (2, 2, 4, 8):
            res.append((q, s))
            q += s
    while q < P:
        rem = P - q
        if ib == NBLK - 1 and rem == 16:
            for s in (8, 4, 2, 2):
                res.append((q, s))
                q += s
            break
        c = min(rem, STACK)
        if rem > 16 and rem - c < 16:
            c = rem - 16
        res.append((q, c))
        q += c
    return res


def _build_program():
    nc = bacc.Bacc(
        "TRN2", target_bir_lowering=False, debug=False, num_devices=NCORES
    )
    Bk = nc.dram_tensor("Bk", [N, P], F32, kind="ExternalInput")
    WT = nc.dram_tensor("WT", [P, P], F32, kind="ExternalInput")
    BkT = nc.dram_tensor("BkT", [P, N], F32, kind="ExternalInput")
    vh = nc.dram_tensor("vh", [P, 4 * P], F32, kind="ExternalInput")
    # per-block masks for the diagonal-square zeroing (1 everywhere except
    # 0 on each group's diagonal square)
    mq = nc.dram_tensor("mq", [NBLK, P, P], F32, kind="ExternalInput")
    out = nc.dram_tensor("out", [NQ, P], F32, kind="ExternalOutput")

    with tile.TileContext(nc) as tc, ExitStack() as ctx:
        consts = ctx.enter_context(tc.tile_pool(name="consts", bufs=1))
        work = ctx.enter_context(tc.tile_pool(name="work", bufs=2))
        small = ctx.enter_context(tc.tile_pool(name="small", bufs=4))
        psum = ctx.enter_context(tc.tile_pool(name="psum", bufs=2, space="PSUM"))

        # preload the exp_and_others ACT table set (covers Tanh + Exp) while
        # the input DMAs are still in flight
        warm = consts.tile([P, 1], F32)
        nc.vector.memset(warm, 0.0)
        nc.scalar.activation(warm, warm, mybir.ActivationFunctionType.Tanh)

        ident = consts.tile([P, P], F32)
        make_identity(nc, ident)

        # critical-path DMAs first: wik matmul needs WT and BkT
        WT_sb = consts.tile([P, P], F32)
        nc.sync.dma_start(out=WT_sb, in_=WT[:, :])
        BkT_sb = consts.tile([P, N], F32)
        nc.sync.dma_start(out=BkT_sb, in_=BkT[:, :])
        # non-critical loads go through the gpsimd (SWDGE) queue so they
        # don't delay the two critical DMAs above
        vh_sb = consts.tile([P, 4 * P], F32)
        nc.gpsimd.dma_start(out=vh_sb, in_=vh[:, :])
        # fp32r (tf32-like, 11-bit mantissa) rounded copy of the one-hot
        # buffer; matmuls on pre-rounded operands run at 1 cycle/row.
        # col P holds v_hi (fp32r-representable), col 3P holds v_lo.
        vhr = consts.tile([P, 4 * P], F32R)
        nc.vector.tensor_copy(vhr, vh_sb)

        mq_sb = consts.tile([P, NBLK, P], F32)
        for b_ in range(NBLK):
            nc.gpsimd.dma_start(out=mq_sb[:, b_, :], in_=mq[b_, :, :])

        # Bk_sb[p, jb*128 + d] = Bk[jb*128 + p, d]  (key rows on partitions)
        Bk_sb = consts.tile([P, N], F32)
        for jb in range(4):
            nc.gpsimd.dma_start(
                out=Bk_sb[:, jb * P : (jb + 1) * P], in_=Bk[jb * P : (jb + 1) * P, :]
            )

        # wik[d, n] = sum_e W[d, e] * Bk[n, e]; split halves so the copy
        # overlaps the second matmul
        wik_ps = psum.tile([P, N], F32, tag="S")
        wik_sb = consts.tile([P, N], F32)
        for h in range(2):
            sl = slice(h * N // 2, (h + 1) * N // 2)
            nc.tensor.matmul(
                wik_ps[:, sl], WT_sb, BkT_sb[:, sl], start=True, stop=True
            )
            nc.vector.tensor_copy(wik_sb[:, sl], wik_ps[:, sl])



        # Symmetry within this core's query square S[0:256, 0:256]:
        #  - a group of queries starting at gq computes j in [gq, 512)
        #    directly (group-aligned raggedness keeps fp32r matmul offsets
        #    and sizes even); the first matmul (start=True) zero-fills
        #    everything below, so uncovered cells are exact zeros
        #  - the uncovered part of the diagonal block := transpose of its
        #    copy with the per-group diagonal squares zeroed
        #  - block 1's j in [0, 128) := transpose of block 0's S[:, 128:256]
        S01_sb = None
        for ib in range(NBLK):
            groups = _groups(ib)
            jfull0 = ib * P  # diagonal block's column range start
            S_ps = psum.tile([P, N], F32, tag="S")
            for g0, gsize in groups:
                gq = ib * P + g0  # group's absolute first query
                tin = work.tile([P, STACK * N], F32, tag="tin")
                offs = []
                off = 0
                for t in range(gsize):
                    iq = gq + t
                    e = iq & ~1  # even-aligned ragged start (fp32r needs even)
                    sz = N - e
                    offs.append((off, e, sz))
                    nc.vector.tensor_scalar_add(
                        tin[:, off : off + sz],
                        wik_sb[:, e:N],
                        wik_sb[:, iq : iq + 1],
                    )
                    off += sz
                tth = work.tile([P, STACK * N], F32R, tag="tth", bufs=3)
                nc.scalar.activation(
                    tth[:, :off],
                    tin[:, :off],
                    mybir.ActivationFunctionType.Tanh,
                )
                for t in range(gsize):
                    il = g0 + t
                    toff, e, sz = offs[t]
                    # S[il, e:] += v . tanh tile via a shifted one-hot-
                    # column view of vhr (fp32r matmul: 1 cycle/row)
                    nc.tensor.matmul(
                        S_ps[:, e:N],
                        vhr[:, P - il : 2 * P - il],
                        tth[:, toff : toff + sz],
                        start=(il == 0),
                        stop=(il == P - 1),
                    )

            # mirror the uncovered lower part of the diagonal block:
            # transpose-accumulate a copy whose per-group diagonal squares
            # are zeroed (those cells were computed directly)
            Zd = work.tile([P, P], F32, tag="Zd")
            nc.vector.tensor_mul(Zd, S_ps[:, jfull0 : jfull0 + P], mq_sb[:, ib, :])
            nc.tensor.matmul(
                S_ps[:, jfull0 : jfull0 + P],
                Zd,
                ident,
                is_transpose=True,
                start=False,
                stop=True,
                skip_group_check=True,
            )

            if ib == 0:
                # stash S[0:128, 128:256] for block 1's mirrored columns
                S01_sb = work.tile([P, P], F32, tag="S01")
                nc.vector.tensor_copy(S01_sb, S_ps[:, P : 2 * P])
            else:
                # mirrored block: S[128:256, 0:128] = S01^T
                nc.tensor.transpose(S_ps[:, 0:P], S01_sb, ident)

            # no max-subtraction: |S| <= sum(v) ~ 64, exp stays in f32 range
            E_sb = work.tile([P, N], F32, tag="E")
            rsum = small.tile([P, 1], F32)
            nc.scalar.activation(
                E_sb,
                S_ps,
                mybir.ActivationFunctionType.Exp,
                accum_out=rsum,
            )
            rrec = small.tile([P, 1], F32)
            nc.vector.reciprocal(rrec, rsum)

            ET_ps = psum.tile([P, N], F32, tag="ET")
            for jb in range(4):
                nc.tensor.transpose(
                    ET_ps[:, jb * P : (jb + 1) * P], E_sb[:, jb * P : (jb + 1) * P], ident
                )
            ET_sb = work.tile([P, N], F32, tag="ET_sb")
            nc.vector.tensor_copy(ET_sb, ET_ps)

            C_ps = psum.tile([P, P], F32, tag="C")
            for jb in range(4):
                nc.tensor.matmul(
                    C_ps,
                    ET_sb[:, jb * P : (jb + 1) * P],
                    Bk_sb[:, jb * P : (jb + 1) * P],
                    start=(jb == 0),
                    stop=(jb == 3),
                )
            C_sb = work.tile([P, P], F32, tag="C_sb")
            nc.vector.tensor_scalar_mul(C_sb, C_ps, rrec)
            nc.sync.dma_start(out=out[ib * P : (ib + 1) * P, :], in_=C_sb)

    nc.compile()
    return nc


def kernel(B, W, v):
    global _program, LAST_RESULT
    B = np.ascontiguousarray(np.asarray(B, dtype=np.float32))
    W = np.ascontiguousarray(np.asarray(W, dtype=np.float32))
    v = np.asarray(v, dtype=np.float32).reshape(P)

    if _program is None:
        _program = _build_program()
    nc = _program

    # split v into fp32r-exact hi (11 mantissa bits) + lo parts
    u = v.view(np.uint32)
    v_hi = ((u + 0x800) & np.uint32(0xFFFFF000)).view(np.float32)
    v_lo = v - v_hi
    vh = np.zeros((P, 4 * P), dtype=np.float32)
    vh[:, P] = v_hi
    vh[:, 3 * P] = v_lo

    # mirror-mask for the diagonal block: keep Z[r, c] only where the
    # target cell (c, r) was NOT computed directly, i.e. r < (c & ~1)
    # (direct coverage of query iq starts at j = iq & ~1)
    r_idx = np.arange(P)[:, None]
    c_idx = np.arange(P)[None, :]
    mq = np.broadcast_to(
        (r_idx < (c_idx & ~1)).astype(np.float32), (NBLK, P, P)
    ).copy()

    WT = np.ascontiguousarray(W.T)
    in_maps = []
    for c in range(NCORES):
        b = c // 2
        q0 = (c % 2) * NQ
        Bp = np.ascontiguousarray(np.roll(B[b], -q0, axis=0))
        in_maps.append(
            {
                "Bk": Bp,
                "BkT": np.ascontiguousarray(Bp.T),
                "WT": WT,
                "vh": vh,
                "mq": mq,
            }
        )

    res = run_bass_kernel_spmd(
        nc, in_maps, core_ids=list(range(NCORES)), trace=TRACE
    )
    LAST_RESULT = res

    C = np.empty((NB, N, P), dtype=np.float32)
    for c in range(NCORES):
        b = c // 2
        q0 = (c % 2) * NQ
        C[b, q0 : q0 + NQ] = res.results[c]["out"]
    return C


# revision 52
# speedup vs baseline: 1.0074x; 1.0074x over previous
"""Bahdanau self-attention kernel for Trainium2 (8 NeuronCores, Bass/Tile).

Math (per batch b):
  Wi = B @ W.T                                  [N, D]
  S[i, j]  = sum_d v[d] * tanh(Wi[i,d] + Wi[j,d])   (symmetric)
  A = softmax(S, axis=-1)
  C = A @ B

Shapes: B [4, 512, 128], W [128, 128], v [128].

Sharding: 8 cores; core c handles batch b = c // 2, query rows
q0 = (c % 2) * 256 .. q0 + 255.  Each core receives its batch's rows
ROTATED so that its 256 query rows are rows 0..255 of its local key
matrix (softmax and the attention-weighted sum are invariant to key
order).  W / v are replicated, so one SPMD program serves all cores.

Per-core pipeline (all layouts put D=128 on partitions):
  - wik[d, n] = (W @ Bk^T)[d, n]    via PE transposes + one matmul
  - for each query i: tanh arg is wik + wik[:, i] broadcast along free
    axis.  DVE tensor_scalar_add builds stacked inputs (STACK queries
    per ACT instruction to amortize the ~352-cycle ACT overhead), ACT
    applies tanh at 1 elem/cycle/lane.
  - v-reduction over d (partitions) via PE: lhsT is a shifted view of a
    [128, 256] buffer holding v at column 128 and zeros elsewhere, so
    lhsT(i)[:, m] = v * (m == i); 128 matmuls accumulate S rows into one
    [128, 512] PSUM tile.
  - softmax: DVE reduce_max(negate) -> ACT exp(S - max) with fused
    free-axis accumulate (row sums) -> DVE reciprocal.
  - C: PE-transpose E, then 4 accumulating matmuls against Bk, scale by
    the reciprocal row sums, DMA out.
"""

import numpy as np
from contextlib import ExitStack

import concourse.bacc as bacc
import concourse.mybir as mybir
import concourse.tile as tile
from concourse.bass_utils import run_bass_kernel_spmd
from concourse.masks import make_identity

F32 = mybir.dt.float32
F32R = mybir.dt.float32r
P = 128  # partitions == feature dim D
N = 512  # sequence length per batch
NB = 4  # batches
NCORES = 8
NQ = 256  # queries per core
NBLK = NQ // P  # query blocks of 128 per core
STACK = 16  # queries per ACT instruction

TRACE = False
LAST_RESULT = None  # BassKernelResults of the most recent run (for profiling)

_program = None


def _groups(ib):
    """Group schedule for block ib: list of (local_start, size).

    Sizes fill the STACK*N stack tile (more queries per ACT call as the
    ragged width shrinks), with a small ramp-in on block 0 (so the first
    ACT fires early) and a taper at the end of the last block (so the
    final PE burst before the last exp is short).  All starts/sizes even
    (fp32r matmul offsets must be even).
    """
    res = []
    q = 0
    if ib == 0:
        for s in (2, 2, 4, 8):
            res.append((q, s))
            q += s
    while q < P:
        rem = P - q
        if ib == NBLK - 1 and rem == 16:
            for s in (8, 4, 2, 2):
                res.append((q, s))
                q += s
            break
        c = min(rem, STACK)
        if rem > 16 and rem - c < 16:
            c = rem - 16
        res.append((q, c))
        q += c
    return res


def _build_program():
    nc = bacc.Bacc(
        "TRN2", target_bir_lowering=False, debug=False, num_devices=NCORES
    )
    Bk = nc.dram_tensor("Bk", [N, P], F32, kind="ExternalInput")
    WT = nc.dram_tensor("WT", [P, P], F32, kind="ExternalInput")
    BkT = nc.dram_tensor("BkT", [P, N], F32, kind="ExternalInput")
    vh = nc.dram_tensor("vh", [P, 4 * P], F32, kind="ExternalInput")
    # per-block masks for the diagonal-square zeroing (1 everywhere except
    # 0 on each group's diagonal square)
    mq = nc.dram_tensor("mq", [NBLK, P, P], F32, kind="ExternalInput")
    out = nc.dram_tensor("out", [NQ, P], F32, kind="ExternalOutput")

    with tile.TileContext(nc) as tc, ExitStack() as ctx:
        consts = ctx.enter_context(tc.tile_pool(name="consts", bufs=1))
        work = ctx.enter_context(tc.tile_pool(name="work", bufs=2))
        small = ctx.enter_context(tc.tile_pool(name="small", bufs=4))
        psum = ctx.enter_context(tc.tile_pool(name="psum", bufs=2, space="PSUM"))

        # preload the exp_and_others ACT table set (covers Tanh + Exp) while
        # the input DMAs are still in flight
        warm = consts.tile([P, 1], F32)
        nc.vector.memset(warm, 0.0)
        nc.scalar.activation(warm, warm, mybir.ActivationFunctionType.Tanh)

        ident = consts.tile([P, P], F32)
        make_identity(nc, ident)

        # critical-path DMAs first: wik matmul needs WT and BkT
        WT_sb = consts.tile([P, P], F32)
        nc.sync.dma_start(out=WT_sb, in_=WT[:, :])
        BkT_sb = consts.tile([P, N], F32)
        nc.sync.dma_start(out=BkT_sb, in_=BkT[:, :])
        vh_sb = consts.tile([P, 4 * P], F32)
        nc.sync.dma_start(out=vh_sb, in_=vh[:, :])
        # fp32r (tf32-like, 11-bit mantissa) rounded copy of the one-hot
        # buffer; matmuls on pre-rounded operands run at 1 cycle/row.
        # col P holds v_hi (fp32r-representable), col 3P holds v_lo.
        vhr = consts.tile([P, 4 * P], F32R)
        nc.vector.tensor_copy(vhr, vh_sb)

        mq_sb = consts.tile([P, NBLK, P], F32)
        for b_ in range(NBLK):
            nc.sync.dma_start(out=mq_sb[:, b_, :], in_=mq[b_, :, :])

        # Bk_sb[p, jb*128 + d] = Bk[jb*128 + p, d]  (key rows on partitions)
        Bk_sb = consts.tile([P, N], F32)
        for jb in range(4):
            nc.sync.dma_start(
                out=Bk_sb[:, jb * P : (jb + 1) * P], in_=Bk[jb * P : (jb + 1) * P, :]
            )

        # wik[d, n] = sum_e W[d, e] * Bk[n, e]
        wik_ps = psum.tile([P, N], F32, tag="S")
        nc.tensor.matmul(wik_ps, WT_sb, BkT_sb, start=True, stop=True)
        wik_sb = consts.tile([P, N], F32)
        nc.vector.tensor_copy(wik_sb, wik_ps)



        # Symmetry within this core's query square S[0:256, 0:256]:
        #  - a group of queries starting at gq computes j in [gq, 512)
        #    directly (group-aligned raggedness keeps fp32r matmul offsets
        #    and sizes even); the first matmul (start=True) zero-fills
        #    everything below, so uncovered cells are exact zeros
        #  - the uncovered part of the diagonal block := transpose of its
        #    copy with the per-group diagonal squares zeroed
        #  - block 1's j in [0, 128) := transpose of block 0's S[:, 128:256]
        S01_sb = None
        for ib in range(NBLK):
            groups = _groups(ib)
            jfull0 = ib * P  # diagonal block's column range start
            S_ps = psum.tile([P, N], F32, tag="S")
            for g0, gsize in groups:
                gq = ib * P + g0  # group's absolute first query
                tin = work.tile([P, STACK * N], F32, tag="tin")
                offs = []
                off = 0
                for t in range(gsize):
                    iq = gq + t
                    e = iq & ~1  # even-aligned ragged start (fp32r needs even)
                    sz = N - e
                    offs.append((off, e, sz))
                    nc.vector.tensor_scalar_add(
                        tin[:, off : off + sz],
                        wik_sb[:, e:N],
                        wik_sb[:, iq : iq + 1],
                    )
                    off += sz
                tth = work.tile([P, STACK * N], F32R, tag="tth", bufs=3)
                nc.scalar.activation(
                    tth[:, :off],
                    tin[:, :off],
                    mybir.ActivationFunctionType.Tanh,
                )
                for t in range(gsize):
                    il = g0 + t
                    toff, e, sz = offs[t]
                    # S[il, e:] += v . tanh tile via a shifted one-hot-
                    # column view of vhr (fp32r matmul: 1 cycle/row)
                    nc.tensor.matmul(
                        S_ps[:, e:N],
                        vhr[:, P - il : 2 * P - il],
                        tth[:, toff : toff + sz],
                        start=(il == 0),
                        stop=(il == P - 1),
                    )

            # mirror the uncovered lower part of the diagonal block:
            # transpose-accumulate a copy whose per-group diagonal squares
            # are zeroed (those cells were computed directly)
            Zd = work.tile([P, P], F32, tag="Zd")
            nc.vector.tensor_mul(Zd, S_ps[:, jfull0 : jfull0 + P], mq_sb[:, ib, :])
            nc.tensor.matmul(
                S_ps[:, jfull0 : jfull0 + P],
                Zd,
                ident,
                is_transpose=True,
                start=False,
                stop=True,
                skip_group_check=True,
            )

            if ib == 0:
                # stash S[0:128, 128:256] for block 1's mirrored columns
                S01_sb = work.tile([P, P], F32, tag="S01")
                nc.vector.tensor_copy(S01_sb, S_ps[:, P : 2 * P])
            else:
                # mirrored block: S[128:256, 0:128] = S01^T
                nc.tensor.transpose(S_ps[:, 0:P], S01_sb, ident)

            # no max-subtraction: |S| <= sum(v) ~ 64, exp stays in f32 range
            E_sb = work.tile([P, N], F32, tag="E")
            rsum = small.tile([P, 1], F32)
            nc.scalar.activation(
                E_sb,
                S_ps,
                mybir.ActivationFunctionType.Exp,
                accum_out=rsum,
            )
            rrec = small.tile([P, 1], F32)
            nc.vector.reciprocal(rrec, rsum)

            ET_ps = psum.tile([P, N], F32, tag="ET")
            for jb in range(4):
                nc.tensor.transpose(
                    ET_ps[:, jb * P : (jb + 1) * P], E_sb[:, jb * P : (jb + 1) * P], ident
                )
            ET_sb = work.tile([P, N], F32, tag="ET_sb")
            nc.vector.tensor_copy(ET_sb, ET_ps)

            C_ps = psum.tile([P, P], F32, tag="C")
            for jb in range(4):
                nc.tensor.matmul(
                    C_ps,
                    ET_sb[:, jb * P : (jb + 1) * P],
                    Bk_sb[:, jb * P : (jb + 1) * P],
                    start=(jb == 0),
                    stop=(jb == 3),
                )
            C_sb = work.tile([P, P], F32, tag="C_sb")
            nc.vector.tensor_scalar_mul(C_sb, C_ps, rrec)
            nc.sync.dma_start(out=out[ib * P : (ib + 1) * P, :], in_=C_sb)

    nc.compile()
    return nc


def kernel(B, W, v):
    global _program, LAST_RESULT
    B = np.ascontiguousarray(np.asarray(B, dtype=np.float32))
    W = np.ascontiguousarray(np.asarray(W, dtype=np.float32))
    v = np.asarray(v, dtype=np.float32).reshape(P)

    if _program is None:
        _program = _build_program()
    nc = _program

    # split v into fp32r-exact hi (11 mantissa bits) + lo parts
    u = v.view(np.uint32)
    v_hi = ((u + 0x800) & np.uint32(0xFFFFF000)).view(np.float32)
    v_lo = v - v_hi
    vh = np.zeros((P, 4 * P), dtype=np.float32)
    vh[:, P] = v_hi
    vh[:, 3 * P] = v_lo

    # mirror-mask for the diagonal block: keep Z[r, c] only where the
    # target cell (c, r) was NOT computed directly, i.e. r < (c & ~1)
    # (direct coverage of query iq starts at j = iq & ~1)
    r_idx = np.arange(P)[:, None]
    c_idx = np.arange(P)[None, :]
    mq = np.broadcast_to(
        (r_idx < (c_idx & ~1)).astype(np.float32), (NBLK, P, P)
    ).copy()

    WT = np.ascontiguousarray(W.T)
    in_maps = []
    for c in range(NCORES):
        b = c // 2
        q0 = (c % 2) * NQ
        Bp = np.ascontiguousarray(np.roll(B[b], -q0, axis=0))
        in_maps.append(
            {
                "Bk": Bp,
                "BkT": np.ascontiguousarray(Bp.T),
                "WT": WT,
                "vh": vh,
                "mq": mq,
            }
        )

    res = run_bass_kernel_spmd(
        nc, in_maps, core_ids=list(range(NCORES)), trace=TRACE
    )
    LAST_RESULT = res

    C = np.empty((NB, N, P), dtype=np.float32)
    for c in range(NCORES):
        b = c // 2
        q0 = (c % 2) * NQ
        C[b, q0 : q0 + NQ] = res.results[c]["out"]
    return C


# revision 53
# speedup vs baseline: 1.0118x; 1.0044x over previous
"""Bahdanau self-attention kernel for Trainium2 (8 NeuronCores, Bass/Tile).

Math (per batch b):
  Wi = B @ W.T                                  [N, D]
  S[i, j]  = sum_d v[d] * tanh(Wi[i,d] + Wi[j,d])   (symmetric)
  A = softmax(S, axis=-1)
  C = A @ B

Shapes: B [4, 512, 128], W [128, 128], v [128].

Sharding: 8 cores; core c handles batch b = c // 2, query rows
q0 = (c % 2) * 256 .. q0 + 255.  Each core receives its batch's rows
ROTATED so that its 256 query rows are rows 0..255 of its local key
matrix (softmax and the attention-weighted sum are invariant to key
order).  W / v are replicated, so one SPMD program serves all cores.

Per-core pipeline (all layouts put D=128 on partitions):
  - wik[d, n] = (W @ Bk^T)[d, n]    via PE transposes + one matmul
  - for each query i: tanh arg is wik + wik[:, i] broadcast along free
    axis.  DVE tensor_scalar_add builds stacked inputs (STACK queries
    per ACT instruction to amortize the ~352-cycle ACT overhead), ACT
    applies tanh at 1 elem/cycle/lane.
  - v-reduction over d (partitions) via PE: lhsT is a shifted view of a
    [128, 256] buffer holding v at column 128 and zeros elsewhere, so
    lhsT(i)[:, m] = v * (m == i); 128 matmuls accumulate S rows into one
    [128, 512] PSUM tile.
  - softmax: DVE reduce_max(negate) -> ACT exp(S - max) with fused
    free-axis accumulate (row sums) -> DVE reciprocal.
  - C: PE-transpose E, then 4 accumulating matmuls against Bk, scale by
    the reciprocal row sums, DMA out.
"""

import numpy as np
from contextlib import ExitStack

import concourse.bacc as bacc
import concourse.mybir as mybir
import concourse.tile as tile
from concourse.bass_utils import run_bass_kernel_spmd
from concourse.masks import make_identity

F32 = mybir.dt.float32
F32R = mybir.dt.float32r
P = 128  # partitions == feature dim D
N = 512  # sequence length per batch
NB = 4  # batches
NCORES = 8
NQ = 256  # queries per core
NBLK = NQ // P  # query blocks of 128 per core
STACK = 16  # queries per ACT instruction

TRACE = False
LAST_RESULT = None  # BassKernelResults of the most recent run (for profiling)

_program = None


def _groups(ib):
    """Group schedule for block ib: list of (local_start, size).

    Sizes fill the STACK*N stack tile (more queries per ACT call as the
    ragged width shrinks), with a small ramp-in on block 0 (so the first
    ACT fires early) and a taper at the end of the last block (so the
    final PE burst before the last exp is short).  All starts/sizes even
    (fp32r matmul offsets must be even).
    """
    res = []
    q = 0
    if ib == 0:
        for s in (2, 2, 4, 8):
            res.append((q, s))
            q += s
    while q < P:
        rem = P - q
        if ib == NBLK - 1 and rem == 16:
            for s in (8, 4, 2, 2):
                res.append((q, s))
                q += s
            break
        c = min(rem, STACK)
        if rem > 16 and rem - c < 16:
            c = rem - 16
        res.append((q, c))
        q += c
    return res


def _build_program():
    nc = bacc.Bacc(
        "TRN2", target_bir_lowering=False, debug=False, num_devices=NCORES
    )
    Bk = nc.dram_tensor("Bk", [N, P], F32, kind="ExternalInput")
    WT = nc.dram_tensor("WT", [P, P], F32, kind="ExternalInput")
    BkT = nc.dram_tensor("BkT", [P, N], F32, kind="ExternalInput")
    vh = nc.dram_tensor("vh", [P, 4 * P], F32, kind="ExternalInput")
    # per-block masks for the diagonal-square zeroing (1 everywhere except
    # 0 on each group's diagonal square)
    mq = nc.dram_tensor("mq", [NBLK, P, P], F32, kind="ExternalInput")
    out = nc.dram_tensor("out", [NQ, P], F32, kind="ExternalOutput")

    with tile.TileContext(nc) as tc, ExitStack() as ctx:
        consts = ctx.enter_context(tc.tile_pool(name="consts", bufs=1))
        work = ctx.enter_context(tc.tile_pool(name="work", bufs=2))
        small = ctx.enter_context(tc.tile_pool(name="small", bufs=4))
        psum = ctx.enter_context(tc.tile_pool(name="psum", bufs=2, space="PSUM"))

        # preload the exp_and_others ACT table set (covers Tanh + Exp) while
        # the input DMAs are still in flight
        warm = consts.tile([P, 1], F32)
        nc.vector.memset(warm, 0.0)
        nc.scalar.activation(warm, warm, mybir.ActivationFunctionType.Tanh)

        ident = consts.tile([P, P], F32)
        make_identity(nc, ident)

        # critical-path DMAs first: wik matmul needs BkT (big, gating) + WT
        BkT_sb = consts.tile([P, N], F32)
        nc.sync.dma_start(out=BkT_sb, in_=BkT[:, :])
        WT_sb = consts.tile([P, P], F32)
        nc.sync.dma_start(out=WT_sb, in_=WT[:, :])
        vh_sb = consts.tile([P, 4 * P], F32)
        nc.sync.dma_start(out=vh_sb, in_=vh[:, :])
        # fp32r (tf32-like, 11-bit mantissa) rounded copy of the one-hot
        # buffer; matmuls on pre-rounded operands run at 1 cycle/row.
        # col P holds v_hi (fp32r-representable), col 3P holds v_lo.
        vhr = consts.tile([P, 4 * P], F32R)
        nc.vector.tensor_copy(vhr, vh_sb)

        mq_sb = consts.tile([P, NBLK, P], F32)
        for b_ in range(NBLK):
            nc.sync.dma_start(out=mq_sb[:, b_, :], in_=mq[b_, :, :])

        # Bk_sb[p, jb*128 + d] = Bk[jb*128 + p, d]  (key rows on partitions)
        Bk_sb = consts.tile([P, N], F32)
        for jb in range(4):
            nc.sync.dma_start(
                out=Bk_sb[:, jb * P : (jb + 1) * P], in_=Bk[jb * P : (jb + 1) * P, :]
            )

        # wik[d, n] = sum_e W[d, e] * Bk[n, e]
        wik_ps = psum.tile([P, N], F32, tag="S")
        nc.tensor.matmul(wik_ps, WT_sb, BkT_sb, start=True, stop=True)
        wik_sb = consts.tile([P, N], F32)
        nc.vector.tensor_copy(wik_sb, wik_ps)



        # Symmetry within this core's query square S[0:256, 0:256]:
        #  - a group of queries starting at gq computes j in [gq, 512)
        #    directly (group-aligned raggedness keeps fp32r matmul offsets
        #    and sizes even); the first matmul (start=True) zero-fills
        #    everything below, so uncovered cells are exact zeros
        #  - the uncovered part of the diagonal block := transpose of its
        #    copy with the per-group diagonal squares zeroed
        #  - block 1's j in [0, 128) := transpose of block 0's S[:, 128:256]
        S01_sb = None
        for ib in range(NBLK):
            groups = _groups(ib)
            jfull0 = ib * P  # diagonal block's column range start
            S_ps = psum.tile([P, N], F32, tag="S")
            for g0, gsize in groups:
                gq = ib * P + g0  # group's absolute first query
                tin = work.tile([P, STACK * N], F32, tag="tin")
                offs = []
                off = 0
                for t in range(gsize):
                    iq = gq + t
                    e = iq & ~1  # even-aligned ragged start (fp32r needs even)
                    sz = N - e
                    offs.append((off, e, sz))
                    nc.vector.tensor_scalar_add(
                        tin[:, off : off + sz],
                        wik_sb[:, e:N],
                        wik_sb[:, iq : iq + 1],
                    )
                    off += sz
                tth = work.tile([P, STACK * N], F32R, tag="tth", bufs=3)
                nc.scalar.activation(
                    tth[:, :off],
                    tin[:, :off],
                    mybir.ActivationFunctionType.Tanh,
                )
                for t in range(gsize):
                    il = g0 + t
                    toff, e, sz = offs[t]
                    # S[il, e:] += v . tanh tile via a shifted one-hot-
                    # column view of vhr (fp32r matmul: 1 cycle/row)
                    nc.tensor.matmul(
                        S_ps[:, e:N],
                        vhr[:, P - il : 2 * P - il],
                        tth[:, toff : toff + sz],
                        start=(il == 0),
                        stop=(il == P - 1),
                    )

            # mirror the uncovered lower part of the diagonal block:
            # transpose-accumulate a copy whose per-group diagonal squares
            # are zeroed (those cells were computed directly)
            Zd = work.tile([P, P], F32, tag="Zd")
            nc.vector.tensor_mul(Zd, S_ps[:, jfull0 : jfull0 + P], mq_sb[:, ib, :])
            nc.tensor.matmul(
                S_ps[:, jfull0 : jfull0 + P],
                Zd,
                ident,
                is_transpose=True,
                start=False,
                stop=True,
                skip_group_check=True,
            )

            if ib == 0:
                # stash S[0:128, 128:256] for block 1's mirrored columns
                S01_sb = work.tile([P, P], F32, tag="S01")
                nc.vector.tensor_copy(S01_sb, S_ps[:, P : 2 * P])
            else:
                # mirrored block: S[128:256, 0:128] = S01^T
                nc.tensor.transpose(S_ps[:, 0:P], S01_sb, ident)

            # no max-subtraction: |S| <= sum(v) ~ 64, exp stays in f32 range
            E_sb = work.tile([P, N], F32, tag="E")
            rsum = small.tile([P, 1], F32)
            nc.scalar.activation(
                E_sb,
                S_ps,
                mybir.ActivationFunctionType.Exp,
                accum_out=rsum,
            )
            rrec = small.tile([P, 1], F32)
            nc.vector.reciprocal(rrec, rsum)

            ET_ps = psum.tile([P, N], F32, tag="ET")
            for jb in range(4):
                nc.tensor.transpose(
                    ET_ps[:, jb * P : (jb + 1) * P], E_sb[:, jb * P : (jb + 1) * P], ident
                )
            ET_sb = work.tile([P, N], F32, tag="ET_sb")
            nc.vector.tensor_copy(ET_sb, ET_ps)

            C_ps = psum.tile([P, P], F32, tag="C")
            for jb in range(4):
                nc.tensor.matmul(
                    C_ps,
                    ET_sb[:, jb * P : (jb + 1) * P],
                    Bk_sb[:, jb * P : (jb + 1) * P],
                    start=(jb == 0),
                    stop=(jb == 3),
                )
            C_sb = work.tile([P, P], F32, tag="C_sb")
            nc.vector.tensor_scalar_mul(C_sb, C_ps, rrec)
            nc.sync.dma_start(out=out[ib * P : (ib + 1) * P, :], in_=C_sb)

    nc.compile()
    return nc


def kernel(B, W, v):
    global _program, LAST_RESULT
    B = np.ascontiguousarray(np.asarray(B, dtype=np.float32))
    W = np.ascontiguousarray(np.asarray(W, dtype=np.float32))
    v = np.asarray(v, dtype=np.float32).reshape(P)

    if _program is None:
        _program = _build_program()
    nc = _program

    # split v into fp32r-exact hi (11 mantissa bits) + lo parts
    u = v.view(np.uint32)
    v_hi = ((u + 0x800) & np.uint32(0xFFFFF000)).view(np.float32)
    v_lo = v - v_hi
    vh = np.zeros((P, 4 * P), dtype=np.float32)
    vh[:, P] = v_hi
    vh[:, 3 * P] = v_lo

    # mirror-mask for the diagonal block: keep Z[r, c] only where the
    # target cell (c, r) was NOT computed directly, i.e. r < (c & ~1)
    # (direct coverage of query iq starts at j = iq & ~1)
    r_idx = np.arange(P)[:, None]
    c_idx = np.arange(P)[None, :]
    mq = np.broadcast_to(
        (r_idx < (c_idx & ~1)).astype(np.float32), (NBLK, P, P)
    ).copy()

    WT = np.ascontiguousarray(W.T)
    in_maps = []
    for c in range(NCORES):
        b = c // 2
        q0 = (c % 2) * NQ
        Bp = np.ascontiguousarray(np.roll(B[b], -q0, axis=0))
        in_maps.append(
            {
                "Bk": Bp,
                "BkT": np.ascontiguousarray(Bp.T),
                "WT": WT,
                "vh": vh,
                "mq": mq,
            }
        )

    res = run_bass_kernel_spmd(
        nc, in_maps, core_ids=list(range(NCORES)), trace=TRACE
    )
    LAST_RESULT = res

    C = np.empty((NB, N, P), dtype=np.float32)
    for c in range(NCORES):
        b = c // 2
        q0 = (c % 2) * NQ
        C[b, q0 : q0 + NQ] = res.results[c]["out"]
    return C
